# revision 6
# baseline (speedup 1.0000x reference)
"""RNN-T Joiner kernel for Trainium2 (Bass/Tile), 8-core data-parallel over batch.

out[b,t,u,v] = (enc[b,t] @ We)[v] + (pred[b,u] @ Wp)[v] + bias[v]

Layout trick: V on partitions, (u, t) on the free dim. Then for fixed u the
pred term is a per-partition scalar, so the broadcast-add is a single-stream
tensor_scalar (DVE, 2x bf16 mode) / tensor_scalar (GpSimd) / activation-with-
bias (Act) — no per-u PE broadcast matmuls, no PSUM in the main loop, and the
elementwise work is split across three engines so the output DMA stream stays
saturated.

W is packed host-side per vocab tile ([vt, d', c, j] with per-partition
contiguous 1280 B runs) so each vt's weights arrive in one full-rate 160 KB
DMA and the first store can issue ~3 us in.

Per core (one batch element):
  - PE (bf16): per vt, pred_projT [v,65] (+bias rank-1) and enc_projT [v,256]
    accumulated in PSUM f32.
  - Act: PSUM->SBUF copy of enc_projT (bf16 cast); DVE: pred_projT copy (f32).
  - DVE/Pool/Act: 65 per-u broadcast adds per vt, bf16 in/out, f32 scalar,
    split 9:4:3 by measured per-instruction cost.
  - HWDGE DMA: 16 stores of ~2.1 MB ([v,u,t]-order bf16 output).

Output returned to DRAM as bf16 [V, U1*T]; host transposes to [T,U1,V] f32.
bf16 end-to-end keeps max rel err ~4e-3, well under the 2e-2 gate.
"""

import sys

sys.path.insert(0, "/opt/trn_rl_repo")

import numpy as np

B, T, U1, D, V = 8, 256, 65, 640, 1024
KC = D // 128   # 5 contraction chunks
NVT = V // 128  # 8 vocab tiles
# u-halves per vocab tile: 2 DMA stores of ~2.1 MB each
HALVES = [(0, 33), (33, 32)]

_COMPILED = None


def _build():
    import concourse.bacc as bacc
    import concourse.tile as tile
    import concourse.mybir as mybir

    f32 = mybir.dt.float32
    bf16 = mybir.dt.bfloat16

    nc = bacc.Bacc("TRN2", target_bir_lowering=False, debug=False, num_devices=8)

    encT = nc.dram_tensor("encT", [D, T], bf16, kind="ExternalInput")
    predT = nc.dram_tensor("predT", [D, U1], bf16, kind="ExternalInput")
    # per-vt packed weights: row vt*128+d' holds W[c*128+d', vt*128+j] at col c*128+j
    Wep = nc.dram_tensor("Wep", [NVT * 128, KC * 128], bf16, kind="ExternalInput")
    Wpp = nc.dram_tensor("Wpp", [NVT * 128, KC * 128], bf16, kind="ExternalInput")
    bias = nc.dram_tensor("bias", [1, V], bf16, kind="ExternalInput")
    ones = nc.dram_tensor("ones", [1, U1], bf16, kind="ExternalInput")
    out = nc.dram_tensor("out", [V, U1 * T], bf16, kind="ExternalOutput")

    with tile.TileContext(nc) as tc:
        with tc.tile_pool(name="consts", bufs=1) as cp:
            predT_sb = []
            for c in range(KC):
                t_ = cp.tile([128, U1], bf16, tag=f"predT{c}")
                nc.sync.dma_start(t_[:], predT[c * 128:(c + 1) * 128, :])
                predT_sb.append(t_)
            bias_sb = cp.tile([1, V], bf16, tag="bias")
            nc.sync.dma_start(bias_sb[:], bias[:])
            ones_sb = cp.tile([1, U1], bf16, tag="ones")
            nc.sync.dma_start(ones_sb[:], ones[:])
            encT_sb = []
            for c in range(KC):
                t_ = cp.tile([128, T], bf16, tag=f"encT{c}")
                nc.sync.dma_start(t_[:], encT[c * 128:(c + 1) * 128, :])
                encT_sb.append(t_)
            Wp_sb = []
            We_sb = []
            for vt in range(NVT):
                t_ = cp.tile([128, KC * 128], bf16, tag=f"Wp{vt}")
                nc.sync.dma_start(t_[:], Wpp[vt * 128:(vt + 1) * 128, :])
                Wp_sb.append(t_)
                t_ = cp.tile([128, KC * 128], bf16, tag=f"We{vt}")
                nc.sync.dma_start(t_[:], Wep[vt * 128:(vt + 1) * 128, :])
                We_sb.append(t_)

            pred_sb = cp.tile([128, NVT * U1], f32, tag="pred_sb")
            enc_sb = cp.tile([128, NVT * T], bf16, tag="enc_sb")

            with tc.tile_pool(name="ppsum", bufs=2, space="PSUM") as pp, \
                 tc.tile_pool(name="epsum", bufs=2, space="PSUM") as ep, \
                 tc.tile_pool(name="stage", bufs=3) as sp:
                for vt in range(NVT):
                    vs = slice(vt * 128, (vt + 1) * 128)
                    # pred_projT[v, u] = pred[u] @ Wp[:, v] + bias[v]
                    psp = pp.tile([128, U1], f32, tag="pp")
                    for c in range(KC):
                        nc.tensor.matmul(
                            psp[:], Wp_sb[vt][:, c * 128:(c + 1) * 128],
                            predT_sb[c][:], start=(c == 0), stop=False)
                    nc.tensor.matmul(
                        psp[:], bias_sb[0:1, vs], ones_sb[0:1, :],
                        start=False, stop=True)
                    nc.vector.tensor_copy(pred_sb[:, vt * U1:(vt + 1) * U1], psp[:])

                    # enc_projT[v, t] = enc[t] @ We[:, v]
                    pse = ep.tile([128, T], f32, tag="pse")
                    for c in range(KC):
                        nc.tensor.matmul(
                            pse[:], We_sb[vt][:, c * 128:(c + 1) * 128],
                            encT_sb[c][:], start=(c == 0), stop=(c == KC - 1))
                    esl = enc_sb[:, vt * T:(vt + 1) * T]
                    nc.scalar.copy(esl, pse[:])

                    # broadcast-add + store
                    for (u0, nu) in HALVES:
                        st = sp.tile([128, 33 * T], bf16, tag="stage")
                        for j in range(nu):
                            u = u0 + j
                            dst = st[:, j * T:(j + 1) * T]
                            sc = pred_sb[:, vt * U1 + u:vt * U1 + u + 1]
                            r = j % 16
                            if r < 9:
                                nc.vector.tensor_scalar_add(dst, esl, sc)
                            elif r < 13:
                                nc.gpsimd.tensor_scalar_add(dst, esl, sc)
                            else:
                                nc.scalar.add(dst, esl, sc)
                        nc.sync.dma_start(
                            out[vs, u0 * T:(u0 + nu) * T], st[:, 0:nu * T])

    nc.compile()
    return nc


def _get_compiled():
    global _COMPILED
    if _COMPILED is None:
        _COMPILED = _build()
    return _COMPILED


def _bf16(a):
    import ml_dtypes
    return np.ascontiguousarray(a.astype(ml_dtypes.bfloat16))


def _pack_w(Whalf):
    # [c*128+d', vt*128+j] -> [vt*128+d', c*128+j]
    w = np.asarray(Whalf).reshape(KC, 128, NVT, 128)
    return w.transpose(2, 1, 0, 3).reshape(NVT * 128, KC * 128)


def _in_maps(encoder_out, predictor_out, W, b):
    W = np.asarray(W)
    Wep = _bf16(_pack_w(W[:D]))
    Wpp = _bf16(_pack_w(W[D:]))
    bias = _bf16(np.asarray(b).reshape(1, V))
    ones = _bf16(np.ones((1, U1), dtype=np.float32))
    maps = []
    for i in range(B):
        maps.append({
            "encT": _bf16(np.asarray(encoder_out[i]).T),
            "predT": _bf16(np.asarray(predictor_out[i]).T),
            "Wep": Wep,
            "Wpp": Wpp,
            "bias": bias,
            "ones": ones,
        })
    return maps


def run(encoder_out, predictor_out, W, b, trace=False, tmpdir=None):
    from concourse.bass_utils import run_bass_kernel_spmd

    nc = _get_compiled()
    maps = _in_maps(encoder_out, predictor_out, W, b)
    res = run_bass_kernel_spmd(
        nc, maps, list(range(B)), trace=trace,
        **({"tmpdir": tmpdir} if tmpdir else {}))
    outs = np.empty((B, T, U1, V), dtype=np.float32)
    for i in range(B):
        o = np.asarray(res.results[i]["out"])
        o16 = o.view(np.uint16).reshape(V, U1, T)
        f = (o16.astype(np.uint32) << np.uint32(16)).view(np.float32)
        outs[i] = f.transpose(2, 1, 0)
    return outs, res


def kernel(encoder_out, predictor_out, W, b):
    outs, _ = run(encoder_out, predictor_out, W, b)
    return outs


# revision 9
# speedup vs baseline: 4.8515x; 4.8515x over previous
"""RNN-T Joiner kernel for Trainium2 (Bass/Tile), 8-core data-parallel over batch.

out[b,t,u,v] = (enc[b,t] @ We)[v] + (pred[b,u] @ Wp)[v] + bias[v]

Layout trick: V on partitions, (u, t) on the free dim. The broadcast add is
done by ONE DVE tensor_tensor per u-half with stride-0 broadcast access
patterns ([128, nu, 32, 8]: enc broadcast over u, pred_rep broadcast over
t-blocks, both with packed 2-byte last dims so the DVE 2x mode stays on).
This amortizes the ~120 ns per-instruction overhead over 8448 elements.
GpSimd is deliberately unused: its tensor_scalar is ~4 us/instr on HW and
its SBUF-port contention degrades DVE ~7x (measured).

W is packed host-side per vocab tile ([vt, d', c, j] with per-partition
contiguous 1280 B runs) so each vt's weights arrive in one full-rate 160 KB
DMA and the first store can issue ~3 us in.

Per core (one batch element):
  - PE (bf16): per vt, pred_projT [v,65] (+bias rank-1) and enc_projT [v,256]
    accumulated in PSUM f32.
  - Act: PSUM->SBUF bf16 copies: enc_projT [128,256] and pred_rep [128,65,8]
    (each pred value replicated 8x so the TT inner dim stays packed).
  - DVE: one tensor_tensor add per u-half (16 total, ~4.4 us each at 2x).
  - HWDGE DMA: 16 stores of ~2.1 MB ([v,u,t]-order bf16 output).

Output returned to DRAM as bf16 [V, U1*T]; host transposes to [T,U1,V] f32.
bf16 end-to-end keeps max rel err ~4e-3, well under the 2e-2 gate.
"""

import sys

sys.path.insert(0, "/opt/trn_rl_repo")

import numpy as np

B, T, U1, D, V = 8, 256, 65, 640, 1024
KC = D // 128   # 5 contraction chunks
NVT = V // 128  # 8 vocab tiles
# u-halves per vocab tile: 2 DMA stores of ~2.1 MB each
HALVES = [(0, 33), (33, 32)]

_COMPILED = None


def _build():
    import concourse.bacc as bacc
    import concourse.tile as tile
    import concourse.mybir as mybir

    f32 = mybir.dt.float32
    bf16 = mybir.dt.bfloat16

    nc = bacc.Bacc("TRN2", target_bir_lowering=False, debug=False, num_devices=8)

    encT = nc.dram_tensor("encT", [D, T], bf16, kind="ExternalInput")
    predT = nc.dram_tensor("predT", [D, U1], bf16, kind="ExternalInput")
    # per-vt packed weights: row vt*128+d' holds W[c*128+d', vt*128+j] at col c*128+j
    Wep = nc.dram_tensor("Wep", [NVT * 128, KC * 128], bf16, kind="ExternalInput")
    Wpp = nc.dram_tensor("Wpp", [NVT * 128, KC * 128], bf16, kind="ExternalInput")
    bias = nc.dram_tensor("bias", [1, V], bf16, kind="ExternalInput")
    ones = nc.dram_tensor("ones", [1, U1], bf16, kind="ExternalInput")
    out = nc.dram_tensor("out", [V, U1 * T], bf16, kind="ExternalOutput")

    with tile.TileContext(nc) as tc:
        with tc.tile_pool(name="consts", bufs=1) as cp:
            predT_sb = []
            for c in range(KC):
                t_ = cp.tile([128, U1], bf16, tag=f"predT{c}")
                nc.sync.dma_start(t_[:], predT[c * 128:(c + 1) * 128, :])
                predT_sb.append(t_)
            bias_sb = cp.tile([1, V], bf16, tag="bias")
            nc.sync.dma_start(bias_sb[:], bias[:])
            ones_sb = cp.tile([1, U1], bf16, tag="ones")
            nc.sync.dma_start(ones_sb[:], ones[:])
            encT_sb = []
            for c in range(KC):
                t_ = cp.tile([128, T], bf16, tag=f"encT{c}")
                nc.sync.dma_start(t_[:], encT[c * 128:(c + 1) * 128, :])
                encT_sb.append(t_)
            Wp_sb = []
            We_sb = []
            for vt in range(NVT):
                t_ = cp.tile([128, KC * 128], bf16, tag=f"Wp{vt}")
                nc.sync.dma_start(t_[:], Wpp[vt * 128:(vt + 1) * 128, :])
                Wp_sb.append(t_)
                t_ = cp.tile([128, KC * 128], bf16, tag=f"We{vt}")
                nc.sync.dma_start(t_[:], Wep[vt * 128:(vt + 1) * 128, :])
                We_sb.append(t_)

            enc_sb = cp.tile([128, NVT * T], bf16, tag="enc_sb")

            with tc.tile_pool(name="ppsum", bufs=2, space="PSUM") as pp, \
                 tc.tile_pool(name="epsum", bufs=2, space="PSUM") as ep, \
                 tc.tile_pool(name="rep", bufs=3) as rp, \
                 tc.tile_pool(name="stage", bufs=3) as sp:
                for vt in range(NVT):
                    vs = slice(vt * 128, (vt + 1) * 128)
                    # pred_projT[v, u] = pred[u] @ Wp[:, v] + bias[v]
                    psp = pp.tile([128, U1], f32, tag="pp")
                    for c in range(KC):
                        nc.tensor.matmul(
                            psp[:], Wp_sb[vt][:, c * 128:(c + 1) * 128],
                            predT_sb[c][:], start=(c == 0), stop=False)
                    nc.tensor.matmul(
                        psp[:], bias_sb[0:1, vs], ones_sb[0:1, :],
                        start=False, stop=True)
                    # pred_rep[v, u, r] = pred_projT[v, u] replicated 8x
                    rep = rp.tile([128, U1 * 8], bf16, tag="rep")
                    nc.scalar.copy(
                        rep[:].rearrange("p (u r) -> p u r", r=8),
                        psp[:].unsqueeze(2).broadcast_to([128, U1, 8]))

                    # enc_projT[v, t] = enc[t] @ We[:, v]
                    pse = ep.tile([128, T], f32, tag="pse")
                    for c in range(KC):
                        nc.tensor.matmul(
                            pse[:], We_sb[vt][:, c * 128:(c + 1) * 128],
                            encT_sb[c][:], start=(c == 0), stop=(c == KC - 1))
                    esl = enc_sb[:, vt * T:(vt + 1) * T]
                    nc.scalar.copy(esl, pse[:])

                    # broadcast-add + store: one TT per u-half
                    for (u0, nu) in HALVES:
                        st = sp.tile([128, 33 * T], bf16, tag="stage")
                        in0 = (esl.rearrange("p (b r) -> p b r", r=8)
                               .unsqueeze(1).broadcast_to([128, nu, 32, 8]))
                        in1 = (rep[:, u0 * 8:(u0 + nu) * 8]
                               .rearrange("p (u r) -> p u r", r=8)
                               .unsqueeze(2).broadcast_to([128, nu, 32, 8]))
                        outp = st[:, 0:nu * T].rearrange(
                            "p (u b r) -> p u b r", u=nu, b=32, r=8)
                        nc.vector.tensor_add(outp, in0, in1)
                        nc.sync.dma_start(
                            out[vs, u0 * T:(u0 + nu) * T], st[:, 0:nu * T])

    nc.compile()
    return nc


def _get_compiled():
    global _COMPILED
    if _COMPILED is None:
        _COMPILED = _build()
    return _COMPILED


def _bf16(a):
    import ml_dtypes
    return np.ascontiguousarray(a.astype(ml_dtypes.bfloat16))


def _pack_w(Whalf):
    # [c*128+d', vt*128+j] -> [vt*128+d', c*128+j]
    w = np.asarray(Whalf).reshape(KC, 128, NVT, 128)
    return w.transpose(2, 1, 0, 3).reshape(NVT * 128, KC * 128)


def _in_maps(encoder_out, predictor_out, W, b):
    W = np.asarray(W)
    Wep = _bf16(_pack_w(W[:D]))
    Wpp = _bf16(_pack_w(W[D:]))
    bias = _bf16(np.asarray(b).reshape(1, V))
    ones = _bf16(np.ones((1, U1), dtype=np.float32))
    maps = []
    for i in range(B):
        maps.append({
            "encT": _bf16(np.asarray(encoder_out[i]).T),
            "predT": _bf16(np.asarray(predictor_out[i]).T),
            "Wep": Wep,
            "Wpp": Wpp,
            "bias": bias,
            "ones": ones,
        })
    return maps


def run(encoder_out, predictor_out, W, b, trace=False, tmpdir=None):
    from concourse.bass_utils import run_bass_kernel_spmd

    nc = _get_compiled()
    maps = _in_maps(encoder_out, predictor_out, W, b)
    res = run_bass_kernel_spmd(
        nc, maps, list(range(B)), trace=trace,
        **({"tmpdir": tmpdir} if tmpdir else {}))
    outs = np.empty((B, T, U1, V), dtype=np.float32)
    for i in range(B):
        o = np.asarray(res.results[i]["out"])
        o16 = o.view(np.uint16).reshape(V, U1, T)
        f = (o16.astype(np.uint32) << np.uint32(16)).view(np.float32)
        outs[i] = f.transpose(2, 1, 0)
    return outs, res


def kernel(encoder_out, predictor_out, W, b):
    outs, _ = run(encoder_out, predictor_out, W, b)
    return outs
